# revision 1
# baseline (speedup 1.0000x reference)
"""Trainium2 Bass kernel for the 2-layer tanh RNN (nn_DeeperRNN).

Strategy
--------
The T=512 recurrence is inherently serial (batch=1), so the program is
replicated on all 8 NeuronCores (identical SPMD program + identical data;
result read from core 0).  The win comes from restructuring:

  phase A:  A1 = X @ W_i2h1.T + b_i2h1 + b_h2h1          (batched matmul)
  phase B:  h1_t = tanh(A1_t + W_h2h1 h1_{t-1})          (512 serial steps)
  phase C:  A2 = H1 @ W_i2h2.T + b_i2h2 + b_h2h2         (batched matmul)
  phase D:  h2_t = tanh(A2_t + W_h2h2 h2_{t-1})          (512 serial steps)
  phase E:  out = h2_T @ W_h2o2.T + b_h2o2

The per-step gemv streams the (bf16) recurrent weight matrix through the
PE as the moving operand with the tiny h vector as the stationary operand,
using 4 column-group tiles (tile_position) for 4 concurrent streams.
The gemv output lands free-major on psum rows {0,32,64,96}; a DVE 32x32
block transpose flips it back to partition-major h-slots, with the j-axis
of every weight matrix host-side permuted so the transpose lands exactly
on the natural slot layout.  tanh runs on ScalarE over the strided
transposed columns.  Biases and the per-step A-term are folded into the
PSUM accumulation as rank-1 matmuls (one-hot / ones stationaries).
"""

import sys
import numpy as np
import ml_dtypes

sys.path.insert(0, "/opt/trn_rl_repo")

import concourse.bass as bass  # noqa: E402
import concourse.mybir as mybir  # noqa: E402
import concourse.bacc as bacc  # noqa: E402
import concourse.tile as tile  # noqa: E402
import concourse.bass_utils as bass_utils  # noqa: E402
from contextlib import ExitStack  # noqa: E402

BF16 = mybir.dt.bfloat16
F32 = mybir.dt.float32
Tanh = mybir.ActivationFunctionType.Tanh

T, IN, H, OUT = 512, 1024, 2048, 1024
NCHUNK = H // 128  # 16


def _host_prep(inputs):
    bf = ml_dtypes.bfloat16
    f32 = np.float32

    def perm_out_axis(a):
        # permute last axis: col (g, J, a2) = g*512 + 32*J + a2 <- row 128J + 32g + a2
        s = a.shape[:-1]
        return np.ascontiguousarray(
            a.reshape(*s, 16, 4, 32).swapaxes(-3, -2).reshape(*s, 2048)
        )

    def prep_wh(w):  # W [j, i] -> [128p, (c*4+g)*512 + J*32 + a2]
        wt = np.asarray(w, f32).T
        return np.ascontiguousarray(
            wt.reshape(16, 128, 16, 4, 32)
            .transpose(1, 0, 3, 2, 4)
            .reshape(128, 16 * 4 * 512)
            .astype(bf)
        )

    def pm(a, part=128):  # [K, N] -> [128, (K//128)*N] chunked partition-major
        k, n = a.shape
        return np.ascontiguousarray(
            a.reshape(k // part, part, n).transpose(1, 0, 2).reshape(part, -1)
        )

    x = np.asarray(inputs["word"], f32).reshape(T, IN)
    return {
        "xt": pm(np.ascontiguousarray(x.T).astype(bf)),
        "w1t": pm(perm_out_axis(np.asarray(inputs["W_i2h1"], f32).T).astype(bf)),
        "wi2t": pm(perm_out_axis(np.asarray(inputs["W_i2h2"], f32).T).astype(bf)),
        "wh1": prep_wh(inputs["W_h2h1"]),
        "wh2": prep_wh(inputs["W_h2h2"]),
        "wo2t": pm(np.asarray(inputs["W_h2o2"], f32).T.astype(bf)),
        "b1": perm_out_axis(
            np.asarray(inputs["b_i2h1"], f32) + np.asarray(inputs["b_h2h1"], f32)
        ).reshape(1, H).astype(bf),
        "b2": perm_out_axis(
            np.asarray(inputs["b_i2h2"], f32) + np.asarray(inputs["b_h2h2"], f32)
        ).reshape(1, H).astype(bf),
        "bo": np.asarray(inputs["b_h2o2"], f32).reshape(1, OUT).astype(bf),
        "ident": np.eye(128, dtype=bf),
        "ones_row": np.ones((1, 128), dtype=bf),
    }


_INPUT_SPECS = {
    "xt": ([128, (IN // 128) * T], BF16),
    "w1t": ([128, (IN // 128) * H], BF16),
    "wi2t": ([128, NCHUNK * H], BF16),
    "wh1": ([128, NCHUNK * 4 * 512], BF16),
    "wh2": ([128, NCHUNK * 4 * 512], BF16),
    "wo2t": ([128, NCHUNK * OUT], BF16),
    "b1": ([1, H], BF16),
    "b2": ([1, H], BF16),
    "bo": ([1, OUT], BF16),
    "ident": ([128, 128], BF16),
    "ones_row": ([1, 128], BF16),
}


def _build(ctx, tc, out_ap, ins):
    nc = tc.nc
    TCH = T // 128

    sb = lambda name, shape, dt: ctx.enter_context(nc.sbuf_tensor(name, shape, dt))

    ident = sb("identsb", [128, 128], BF16)
    nc.sync.dma_start(ident[:], ins["ident"])
    ones_row = sb("onessb", [1, 128], BF16)
    nc.sync.dma_start(ones_row[:], ins["ones_row"])

    a1 = sb("a1sb", [128, TCH * H], BF16)
    a2 = a1  # phases don't overlap: layer-2 A reuses the same buffer
    h1 = sb("h1sb", [128, (T + 1) * 16], BF16)
    h2 = h1  # layer-2 h reuses the same buffer (layer-1 h consumed in phase C)
    nc.vector.memset(h1[:, 0:16], 0.0)

    ts_sb = sb("tssb", [128, 512], F32)  # transpose scratch

    ppool = ctx.enter_context(tc.tile_pool(name="ppool", bufs=2, space="PSUM"))
    bpool = ctx.enter_context(tc.tile_pool(name="bpool", bufs=4, space="PSUM"))

    def batched_proj(a_dst, lhs_of, kchunks, w_sb, bias_sb, tag):
        for tch in range(TCH):
            for ns in range(4):
                pst = bpool.tile([128, 512], F32, tag="pb", name=f"pb_{tag}_{tch}_{ns}")
                ps = pst[0:128, :]
                for kc in range(kchunks):
                    nc.tensor.matmul(
                        ps, lhs_of(kc, tch),
                        w_sb[:, kc * H + ns * 512: kc * H + (ns + 1) * 512],
                        start=(kc == 0), stop=False)
                nc.tensor.matmul(
                    ps, ones_row[:, 0:128], bias_sb[:, ns * 512:(ns + 1) * 512],
                    start=False, stop=True)
                nc.vector.tensor_copy(
                    a_dst[:, tch * H + ns * 512: tch * H + (ns + 1) * 512], ps)

    def recurrence(h_buf, wh_sb, a_sb, tag):
        for t in range(T):
            ps = ppool.tile([128, 512], F32, tag="pz", name=f"pz_{tag}_{t}")
            if t < 2:
                nc.vector.memset(ps[:], 0.0)
            for c in range(17):
                for g in range(4):
                    if c == 0:
                        lhsT = ident[:, t % 128: t % 128 + 1]
                        rhs = a_sb[:, (t // 128) * H + g * 512: (t // 128) * H + (g + 1) * 512]
                    else:
                        cc = c - 1
                        lhsT = h_buf[:, t * 16 + cc: t * 16 + cc + 1]
                        rhs = wh_sb[:, (cc * 4 + g) * 512: (cc * 4 + g + 1) * 512]
                    nc.tensor.matmul(ps[32 * g: 32 * g + 1, :], lhsT, rhs,
                                     start=(c == 0), stop=(c == 16),
                                     tile_position=(0, 32 * g))
            nc.vector.transpose(ts_sb[:], ps[:])
            strided = ts_sb[:].rearrange("p (a b) -> p a b", b=32)[:, :, 0:1]
            nc.scalar.activation(
                h_buf[:, (t + 1) * 16: (t + 2) * 16].unsqueeze(-1), strided, Tanh)

    # ---- phase A ----
    xt_sb = sb("xtsb", [128, (IN // 128) * T], BF16)
    nc.sync.dma_start(xt_sb[:], ins["xt"])
    b1_sb = sb("b1sb", [1, H], BF16)
    nc.sync.dma_start(b1_sb[:], ins["b1"])
    b2_sb = sb("b2sb", [1, H], BF16)
    nc.sync.dma_start(b2_sb[:], ins["b2"])
    bo_sb = sb("bosb", [1, OUT], BF16)
    nc.sync.dma_start(bo_sb[:], ins["bo"])

    wpool = ctx.enter_context(tc.tile_pool(name="wpool", bufs=2))

    w1t_sb = wpool.tile([128, NCHUNK * 4 * 512], BF16, tag="w", name="w1t_t")
    nc.sync.dma_start(w1t_sb[:, 0:(IN // 128) * H], ins["w1t"])
    batched_proj(
        a1, lambda kc, tch: xt_sb[:, kc * T + tch * 128: kc * T + tch * 128 + 128],
        IN // 128, w1t_sb, b1_sb, "a1")

    # ---- phase B ----
    wh1_sb = wpool.tile([128, NCHUNK * 4 * 512], BF16, tag="w", name="wh1_t")
    nc.sync.dma_start(wh1_sb[:], ins["wh1"])
    recurrence(h1, wh1_sb, a1, "l1")

    # ---- phase C ----
    wi2t_sb = wpool.tile([128, NCHUNK * 4 * 512], BF16, tag="w", name="wi2_t")
    nc.sync.dma_start(wi2t_sb[:], ins["wi2t"])
    h1v = h1[:].rearrange("p (t c) -> p t c", c=16)
    batched_proj(
        a2, lambda kc, tch: h1v[:, 1 + tch * 128: 1 + tch * 128 + 128, kc: kc + 1],
        NCHUNK, wi2t_sb, b2_sb, "a2")

    # ---- phase D ----
    wh2_sb = wpool.tile([128, NCHUNK * 4 * 512], BF16, tag="w", name="wh2_t")
    nc.sync.dma_start(wh2_sb[:], ins["wh2"])
    recurrence(h2, wh2_sb, a2, "l2")

    # ---- phase E ----
    wo2t_sb = wpool.tile([128, NCHUNK * 4 * 512], BF16, tag="w", name="wo2_t")
    nc.sync.dma_start(wo2t_sb[:, 0:NCHUNK * OUT], ins["wo2t"])
    out_sb = sb("outsb", [1, OUT], F32)
    for ns in range(2):
        pso = bpool.tile([128, 512], F32, tag="pb", name=f"pso{ns}")
        ps = pso[0:1, :]
        for c in range(NCHUNK):
            nc.tensor.matmul(
                ps, h2[:, T * 16 + c: T * 16 + c + 1],
                wo2t_sb[:, c * OUT + ns * 512: c * OUT + (ns + 1) * 512],
                start=(c == 0), stop=False)
        nc.tensor.matmul(ps, ones_row[:, 0:1], bo_sb[:, ns * 512:(ns + 1) * 512],
                         start=False, stop=True)
        nc.vector.tensor_copy(out_sb[:, ns * 512:(ns + 1) * 512], ps)
    nc.sync.dma_start(out_ap, out_sb[:])


_CACHE = {}


def _get_compiled():
    if "nc" in _CACHE:
        return _CACHE["nc"], _CACHE["in_names"]
    nc = bacc.Bacc("TRN2", target_bir_lowering=False, debug=False, num_devices=8)
    ins = {k: nc.dram_tensor(k, shp, dt, kind="ExternalInput")
           for k, (shp, dt) in _INPUT_SPECS.items()}
    out_dram = nc.dram_tensor("out", [1, OUT], F32, kind="ExternalOutput")
    with tile.TileContext(nc) as tc:
        with ExitStack() as ctx:
            _build(ctx, tc, out_dram.ap(), {k: v.ap() for k, v in ins.items()})
    nc.compile()
    _CACHE["nc"] = nc
    _CACHE["in_names"] = list(ins)
    return nc, list(ins)


def kernel(**inputs) -> np.ndarray:
    prep = _host_prep(inputs)
    nc, in_names = _get_compiled()
    in_map = {k: prep[k] for k in in_names}
    res = bass_utils.run_bass_kernel_spmd(
        nc, [in_map] * 8, core_ids=list(range(8)))
    return np.asarray(res.results[0]["out"], dtype=np.float32)

